# revision 2
# baseline (speedup 1.0000x reference)
"""Multi-head attention (b=2, sq=skv=2048, dim=1024, 16 heads x 64) on 8 TRN2
NeuronCores.

Sharding: 2 heads per core (head-parallel across batch*heads), with the
matching tensor-parallel column slice of W_qkv and row slice of W_out.  Each
core computes a partial output projection over its 128 head-dims; the
all-reduce of the 8 partials (+ bias) happens on the host during unshard.

Per-core kernel (all bf16 compute, fp32 PSUM accumulation):
  phase 1: qT/kT/vT = W.T @ x.T   ([128 = 2 heads x 64 dims, tokens]); v is
           additionally PE-transposed to natural [token, dim] layout with a
           ones column appended (denominator trick).
  phase 2: per (batch, head, q-tile): scoresT = kT.T_slice @ qT_slice in PSUM
           ([k-tokens, q]); exp via ScalarE (scale=1/8 fused, no max
           subtraction -- scores are ~N(0,1), max over all scores ~6); PV
           matmul accumulates [v | 1].T @ expT over the 16 k-tiles, giving
           unnormalized outT plus the softmax denominator in row 64;
           normalize with reciprocal + PE outer-product broadcast.
  phase 3: partial out = outT.T @ W_out_rows -> [tokens, 1024] bf16.
"""

import os
import sys

for _p in ("/opt/trn_rl_repo", "/root/.axon_site/_ro/trn_rl_repo"):
    if os.path.isdir(_p) and _p not in sys.path:
        sys.path.append(_p)

import ml_dtypes
import numpy as np

import concourse.bass as bass  # noqa: F401
import concourse.tile as tile
from concourse import bacc, mybir
from concourse.bass_utils import run_bass_kernel_spmd
from concourse.masks import make_identity

B, SQ, SKV, DIM = 2, 2048, 2048, 1024
HEADS, DH = 16, 64
N_CORES = 8
HPC = HEADS // N_CORES  # heads per core = 2
HD = HPC * DH  # 128 head-dim rows per core
TOK = B * SQ  # 4096
KO = DIM // 128  # 8 contraction chunks of 128
SCALE = DH**-0.5

BF16 = mybir.dt.bfloat16
F32 = mybir.dt.float32

PCHUNK = 512  # token chunk in projections
QTILE = 512  # q tile in attention
KTILE = 128  # k tile (scores psum partition dim)
NKT = SKV // KTILE  # 16
NQT = SQ // QTILE  # 4

BF = ml_dtypes.bfloat16


def build():
    nc = bacc.Bacc(
        "TRN2", target_bir_lowering=False, debug=False, num_devices=N_CORES
    )

    xqt_d = nc.dram_tensor("xqt", [DIM, TOK], BF16, kind="ExternalInput")
    xkvt_d = nc.dram_tensor("xkvt", [DIM, TOK], BF16, kind="ExternalInput")
    wq_d = nc.dram_tensor("wq", [DIM, HD], BF16, kind="ExternalInput")
    wk_d = nc.dram_tensor("wk", [DIM, HD], BF16, kind="ExternalInput")
    wv_d = nc.dram_tensor("wv", [DIM, HD], BF16, kind="ExternalInput")
    wout_d = nc.dram_tensor("wout", [HD, DIM], BF16, kind="ExternalInput")
    out_d = nc.dram_tensor("out", [TOK, DIM], BF16, kind="ExternalOutput")

    xqt = xqt_d.ap().rearrange("(ko p) t -> p ko t", p=128)
    xkvt = xkvt_d.ap().rearrange("(ko p) t -> p ko t", p=128)

    with tile.TileContext(nc) as tc:
        with (
            tc.tile_pool(name="persist", bufs=1) as persist,
            tc.tile_pool(name="xin", bufs=3) as xin,
            tc.tile_pool(name="exps", bufs=4) as exps,
            tc.tile_pool(name="ost", bufs=3) as ost,
            tc.tile_pool(name="spsum", bufs=2, space="PSUM") as spsum,
            tc.tile_pool(name="accp", bufs=2, space="PSUM") as accp,
            tc.tile_pool(name="miscp", bufs=2, space="PSUM") as miscp,
        ):
            # --- weights / constants ---
            wq_sb = persist.tile([128, KO, HD], BF16, tag="wq")
            nc.sync.dma_start(wq_sb[:], wq_d.ap().rearrange("(ko p) m -> p ko m", p=128))
            wk_sb = persist.tile([128, KO, HD], BF16, tag="wk")
            nc.sync.dma_start(wk_sb[:], wk_d.ap().rearrange("(ko p) m -> p ko m", p=128))
            wv_sb = persist.tile([128, KO, HD], BF16, tag="wv")
            nc.sync.dma_start(wv_sb[:], wv_d.ap().rearrange("(ko p) m -> p ko m", p=128))
            wout_sb = persist.tile([HD, DIM], BF16, tag="wout")
            nc.sync.dma_start(wout_sb[:], wout_d.ap())

            ident = persist.tile([128, DH], BF16, tag="ident")
            make_identity(nc, ident[0:DH, :])
            make_identity(nc, ident[DH : 2 * DH, :])
            ones_f32 = persist.tile([1, DH], F32, tag="ones")
            nc.vector.memset(ones_f32[:], 1.0)

            qt_sb, kt_sb, vt_sb, vnat, outT = {}, {}, {}, {}, {}
            for b in range(B):
                qt_sb[b] = persist.tile([HD, SQ], BF16, tag=f"qt{b}", name=f"qt{b}")
                kt_sb[b] = persist.tile([HD, SKV], BF16, tag=f"kt{b}", name=f"kt{b}")
                vt_sb[b] = persist.tile([HD, SKV], BF16, tag=f"vt{b}", name=f"vt{b}")
                vnat[b] = persist.tile([128, HPC, NKT, DH + 1], BF16, tag=f"vn{b}", name=f"vn{b}")
                outT[b] = persist.tile([HD, SQ], BF16, tag=f"ot{b}", name=f"ot{b}")
                nc.vector.memset(vnat[b][:, :, :, DH], 1.0)

            def project(dst, w_sb, x_ap, tok0):
                """dst[:, t] += sum_D w[D, :] x[D, tok0 + t] for SQ tokens."""
                for tt in range(SQ // PCHUNK):
                    xt = xin.tile([128, KO, PCHUNK], BF16, tag="x")
                    nc.sync.dma_start(
                        xt[:],
                        x_ap[:, :, tok0 + tt * PCHUNK : tok0 + (tt + 1) * PCHUNK],
                    )
                    ps = miscp.tile([128, PCHUNK], F32, tag="m")
                    for ko in range(KO):
                        nc.tensor.matmul(
                            ps[:],
                            w_sb[:, ko, :],
                            xt[:, ko, :],
                            start=(ko == 0),
                            stop=(ko == KO - 1),
                        )
                    nc.any.tensor_copy(
                        dst[:, tt * PCHUNK : (tt + 1) * PCHUNK], ps[:]
                    )

            def build_vnat(b):
                for j in range(NKT):
                    for h in range(HPC):
                        tp = miscp.tile([128, DH], BF16, tag="m")
                        nc.tensor.transpose(
                            tp[:],
                            vt_sb[b][
                                h * DH : (h + 1) * DH,
                                j * KTILE : (j + 1) * KTILE,
                            ],
                            ident[h * DH : (h + 1) * DH, :],
                        )
                        nc.any.tensor_copy(vnat[b][:, h, j, 0:DH], tp[:])

            def attention(b):
                for qt in range(NQT):
                    q_sl = slice(qt * QTILE, (qt + 1) * QTILE)
                    accs = [
                        accp.tile([128, QTILE], F32, tag="acc", name="acc") for _ in range(HPC)
                    ]
                    for j in range(NKT):
                        k_sl = slice(j * KTILE, (j + 1) * KTILE)
                        sp = spsum.tile([128, HPC, QTILE], F32, tag="s")
                        for h in range(HPC):
                            h_sl = slice(h * DH, (h + 1) * DH)
                            nc.tensor.matmul(
                                sp[:, h, :],
                                kt_sb[b][h_sl, k_sl],
                                qt_sb[b][h_sl, q_sl],
                                start=True,
                                stop=True,
                            )
                        ex = exps.tile([128, HPC, QTILE], BF16, tag="e")
                        nc.scalar.activation(
                            ex[:],
                            sp[:],
                            mybir.ActivationFunctionType.Exp,
                            scale=SCALE,
                        )
                        for h in range(HPC):
                            nc.tensor.matmul(
                                accs[h][0 : DH + 1, :],
                                vnat[b][:, h, j, :],
                                ex[:, h, :],
                                start=(j == 0),
                                stop=(j == NKT - 1),
                            )
                    for h in range(HPC):
                        h_sl = slice(h * DH, (h + 1) * DH)
                        recip = ost.tile([1, QTILE], F32, tag="r")
                        nc.vector.reciprocal(recip[:], accs[h][DH : DH + 1, :])
                        bc = miscp.tile([DH, QTILE], F32, tag="m")
                        nc.tensor.matmul(
                            bc[:], ones_f32[:], recip[:], start=True, stop=True
                        )
                        bcs = ost.tile([DH, QTILE], F32, tag="bc")
                        nc.any.tensor_copy(bcs[:], bc[:])
                        nc.vector.tensor_mul(
                            outT[b][h_sl, q_sl], accs[h][0:DH, :], bcs[:]
                        )

            def outproj(b):
                for tt in range(SQ // 128):
                    t_sl = slice(tt * 128, (tt + 1) * 128)
                    for nt in range(DIM // 512):
                        ps = miscp.tile([128, 512], F32, tag="m")
                        nc.tensor.matmul(
                            ps[:],
                            outT[b][:, t_sl],
                            wout_sb[:, nt * 512 : (nt + 1) * 512],
                            start=True,
                            stop=True,
                        )
                        ob = ost.tile([128, 512], BF16, tag="o")
                        nc.any.tensor_copy(ob[:], ps[:])
                        nc.sync.dma_start(
                            out_d.ap()[
                                b * SQ + tt * 128 : b * SQ + (tt + 1) * 128,
                                nt * 512 : (nt + 1) * 512,
                            ],
                            ob[:],
                        )

            for b in range(B):
                project(kt_sb[b], wk_sb, xkvt, b * SKV)
                project(vt_sb[b], wv_sb, xkvt, b * SKV)
                build_vnat(b)
                project(qt_sb[b], wq_sb, xqt, b * SQ)
                attention(b)
                outproj(b)

    nc.compile()
    return nc


def make_in_maps(x_q, x_kv, W_qkv, W_out):
    x_q = np.asarray(x_q, dtype=np.float32)
    x_kv = np.asarray(x_kv, dtype=np.float32)
    W_qkv = np.asarray(W_qkv, dtype=np.float32)
    W_out = np.asarray(W_out, dtype=np.float32)

    xqt = np.ascontiguousarray(x_q.reshape(TOK, DIM).T).astype(BF)
    xkvt = np.ascontiguousarray(x_kv.reshape(TOK, DIM).T).astype(BF)

    in_maps = []
    for c in range(N_CORES):
        cs = slice(c * HD, (c + 1) * HD)
        in_maps.append(
            {
                "xqt": xqt,
                "xkvt": xkvt,
                "wq": np.ascontiguousarray(W_qkv[:, cs]).astype(BF),
                "wk": np.ascontiguousarray(W_qkv[:, 1024:][:, cs]).astype(BF),
                "wv": np.ascontiguousarray(W_qkv[:, 2048:][:, cs]).astype(BF),
                "wout": np.ascontiguousarray(W_out[cs, :]).astype(BF),
            }
        )
    return in_maps


def combine(partials, b_out):
    """Sum the 8 per-core partial projections and add the bias."""
    acc = np.zeros((TOK, DIM), dtype=np.float32)
    for p in partials:
        acc += np.asarray(p, dtype=np.float32)
    acc += np.asarray(b_out, dtype=np.float32)
    return acc.reshape(B, SQ, DIM)


_STATE = {}


def _get_nc():
    if "nc" not in _STATE:
        _STATE["nc"] = build()
    return _STATE["nc"]


def run(x_q, x_kv, W_qkv, W_out, b_out, trace=False):
    nc = _get_nc()
    in_maps = make_in_maps(x_q, x_kv, W_qkv, W_out)
    res = run_bass_kernel_spmd(nc, in_maps, list(range(N_CORES)), trace=trace)
    out = combine([r["out"] for r in res.results], b_out)
    return out, res


def kernel(x_q, x_kv, W_qkv, W_out, b_out):
    out, _ = run(x_q, x_kv, W_qkv, W_out, b_out, trace=False)
    return out


# revision 5
# speedup vs baseline: 1.0181x; 1.0181x over previous
"""Multi-head attention (b=2, sq=skv=2048, dim=1024, 16 heads x 64) on 8 TRN2
NeuronCores.

Sharding: 2 heads per core (head-parallel across batch*heads), with the
matching tensor-parallel column slice of W_qkv and row slice of W_out.  Each
core computes a partial output projection over its 128 head-dims; the
all-reduce of the 8 partials (+ bias) happens on the host during unshard.

Per-core kernel (bf16 compute, fp32 PSUM accumulation):
  phase 1: qT/kT/vT = W.T @ x.T   ([128 = 2 heads x 64 dims, tokens]); v is
           additionally PE-transposed to natural [token, dim] layout with a
           ones column appended (denominator trick).
  phase 2: per (batch, q-tile, k-tile): scoresT for both heads ([k-tokens, q])
           in one 2-bank PSUM group; one exp ACTIVATE over the group (scale
           1/8 fused, no max subtraction -- scores range +-10); PV matmuls
           accumulate [v | 1].T @ expT over the 16 k-tiles giving unnormalized
           outT plus the softmax denominator in row 64.  The accumulator is
           copied to SBUF immediately (releasing PSUM); normalization
           (reciprocal + PE outer-product broadcast + multiply) happens off
           the critical path.
  phase 3: partial out = outT.T @ W_out_rows, DMA'd straight from PSUM to
           DRAM as fp32 [tokens, 1024].
"""

import os
import sys

for _p in ("/opt/trn_rl_repo", "/root/.axon_site/_ro/trn_rl_repo"):
    if os.path.isdir(_p) and _p not in sys.path:
        sys.path.append(_p)

import ml_dtypes
import numpy as np

import concourse.bass as bass  # noqa: F401
import concourse.tile as tile
from concourse import bacc, mybir
from concourse.bass_utils import run_bass_kernel_spmd
from concourse.masks import make_identity

B, SQ, SKV, DIM = 2, 2048, 2048, 1024
HEADS, DH = 16, 64
N_CORES = 8
HPC = HEADS // N_CORES  # heads per core = 2
HD = HPC * DH  # 128 head-dim rows per core
TOK = B * SQ  # 4096
KO = DIM // 128  # 8 contraction chunks of 128
SCALE = DH**-0.5

BF16 = mybir.dt.bfloat16
F32 = mybir.dt.float32

PCHUNK = 1024  # token chunk in projections (2KB dram lines)
QTILE = 512  # q tile in attention
KTILE = 128  # k tile (scores psum partition dim)
NKT = SKV // KTILE  # 16
NQT = SQ // QTILE  # 4

BF = ml_dtypes.bfloat16
Exp = mybir.ActivationFunctionType.Exp


def build():
    nc = bacc.Bacc(
        "TRN2", target_bir_lowering=False, debug=False, num_devices=N_CORES
    )

    xqt_d = nc.dram_tensor("xqt", [DIM, TOK], BF16, kind="ExternalInput")
    xkvt_d = nc.dram_tensor("xkvt", [DIM, TOK], BF16, kind="ExternalInput")
    wq_d = nc.dram_tensor("wq", [DIM, HD], BF16, kind="ExternalInput")
    wk_d = nc.dram_tensor("wk", [DIM, HD], BF16, kind="ExternalInput")
    wv_d = nc.dram_tensor("wv", [DIM, HD], BF16, kind="ExternalInput")
    wout_d = nc.dram_tensor("wout", [HD, DIM], BF16, kind="ExternalInput")
    out_d = nc.dram_tensor("out", [TOK, DIM], BF16, kind="ExternalOutput")

    xqt = xqt_d.ap().rearrange("(ko p) t -> p ko t", p=128)
    xkvt = xkvt_d.ap().rearrange("(ko p) t -> p ko t", p=128)

    with tile.TileContext(nc) as tc:
        with (
            tc.tile_pool(name="persist", bufs=1) as persist,
            tc.tile_pool(name="xin", bufs=3) as xin,
            tc.tile_pool(name="exps", bufs=6) as exps,
            tc.tile_pool(name="ost", bufs=3) as ost,
            tc.tile_pool(name="spsum", bufs=2, space="PSUM") as spsum,
            tc.tile_pool(name="accp", bufs=2, space="PSUM") as accp,
            tc.tile_pool(name="miscp", bufs=2, space="PSUM") as miscp,
        ):
            # --- weights / constants ---
            wq_sb = persist.tile([128, KO, HD], BF16, tag="wq")
            nc.sync.dma_start(wq_sb[:], wq_d.ap().rearrange("(ko p) m -> p ko m", p=128))
            wk_sb = persist.tile([128, KO, HD], BF16, tag="wk")
            nc.sync.dma_start(wk_sb[:], wk_d.ap().rearrange("(ko p) m -> p ko m", p=128))
            wv_sb = persist.tile([128, KO, HD], BF16, tag="wv")
            nc.sync.dma_start(wv_sb[:], wv_d.ap().rearrange("(ko p) m -> p ko m", p=128))
            wout_sb = persist.tile([HD, DIM], BF16, tag="wout")
            nc.sync.dma_start(wout_sb[:], wout_d.ap())

            ident = persist.tile([128, DH], BF16, tag="ident")
            make_identity(nc, ident[0:DH, :])
            make_identity(nc, ident[DH : 2 * DH, :])
            ones_f32 = persist.tile([1, DH], F32, tag="ones")
            nc.vector.memset(ones_f32[:], 1.0)

            qt_sb, kt_sb, vt_sb, vnat, outT = {}, {}, {}, {}, {}
            for b in range(B):
                qt_sb[b] = persist.tile([HD, SQ], BF16, tag=f"qt{b}", name=f"qt{b}")
                kt_sb[b] = persist.tile([HD, SKV], BF16, tag=f"kt{b}", name=f"kt{b}")
                vt_sb[b] = persist.tile([HD, SKV], BF16, tag=f"vt{b}", name=f"vt{b}")
                vnat[b] = persist.tile(
                    [128, HPC, NKT, DH + 1], BF16, tag=f"vn{b}", name=f"vn{b}"
                )
                outT[b] = persist.tile([HD, SQ], BF16, tag=f"ot{b}", name=f"ot{b}")
                nc.vector.memset(vnat[b][:, :, :, DH], 1.0)

            def project(dst, w_sb, x_ap, tok0):
                """dst[:, t] = sum_D w[D, :] x[D, tok0 + t] for SQ tokens."""
                for tt in range(SQ // PCHUNK):
                    xt = xin.tile([128, KO, PCHUNK], BF16, tag="x")
                    nc.sync.dma_start(
                        xt[:],
                        x_ap[:, :, tok0 + tt * PCHUNK : tok0 + (tt + 1) * PCHUNK],
                    )
                    for sub in range(PCHUNK // 512):
                        ps = miscp.tile([128, 512], F32, tag="m", name="projp")
                        for ko in range(KO):
                            nc.tensor.matmul(
                                ps[:],
                                w_sb[:, ko, :],
                                xt[:, ko, sub * 512 : (sub + 1) * 512],
                                start=(ko == 0),
                                stop=(ko == KO - 1),
                            )
                        t0 = tt * PCHUNK + sub * 512
                        nc.vector.tensor_copy(dst[:, t0 : t0 + 512], ps[:])

            def build_vnat(b):
                for jg in range(NKT // 4):
                    for h in range(HPC):
                        tp = miscp.tile([128, 4, DH], BF16, tag="m", name="vtp")
                        for i in range(4):
                            j = jg * 4 + i
                            nc.tensor.transpose(
                                tp[:, i, :],
                                vt_sb[b][
                                    h * DH : (h + 1) * DH,
                                    j * KTILE : (j + 1) * KTILE,
                                ],
                                ident[h * DH : (h + 1) * DH, :],
                            )
                        nc.vector.tensor_copy(
                            vnat[b][:, h, jg * 4 : (jg + 1) * 4, 0:DH], tp[:]
                        )

            def attention(b):
                for qt in range(NQT):
                    q_sl = slice(qt * QTILE, (qt + 1) * QTILE)
                    accs = [
                        accp.tile([128, QTILE], F32, tag="acc", name="acc")
                        for _ in range(HPC)
                    ]
                    for j in range(NKT):
                        k_sl = slice(j * KTILE, (j + 1) * KTILE)
                        sp = spsum.tile([128, HPC, QTILE], F32, tag="s")
                        for h in range(HPC):
                            h_sl = slice(h * DH, (h + 1) * DH)
                            nc.tensor.matmul(
                                sp[:, h, :],
                                kt_sb[b][h_sl, k_sl],
                                qt_sb[b][h_sl, q_sl],
                                start=True,
                                stop=True,
                            )
                        ex = exps.tile([128, HPC, QTILE], BF16, tag="e")
                        nc.scalar.activation(ex[:], sp[:], Exp, scale=SCALE)
                        for h in range(HPC):
                            nc.tensor.matmul(
                                accs[h][0 : DH + 1, :],
                                vnat[b][:, h, j, :],
                                ex[:, h, :],
                                start=(j == 0),
                                stop=(j == NKT - 1),
                            )
                    for h in range(HPC):
                        h_sl = slice(h * DH, (h + 1) * DH)
                        # copy unnormalized out + denominator to SBUF, freeing
                        # the PSUM accumulator immediately
                        u = ost.tile([DH + 1, QTILE], F32, tag="u")
                        nc.vector.tensor_copy(u[:], accs[h][0 : DH + 1, :])
                        recip = ost.tile([1, QTILE], F32, tag="r")
                        nc.vector.reciprocal(recip[:], u[DH : DH + 1, :])
                        bc = miscp.tile([DH, QTILE], F32, tag="m", name="bc")
                        nc.tensor.matmul(
                            bc[:], ones_f32[:], recip[:], start=True, stop=True
                        )
                        nc.vector.tensor_mul(
                            outT[b][h_sl, q_sl], u[0:DH, :], bc[:]
                        )

            def outproj(b):
                for tt in range(SQ // 128):
                    t_sl = slice(tt * 128, (tt + 1) * 128)
                    for nt in range(DIM // 512):
                        ps = miscp.tile([128, 512], F32, tag="m", name="projo")
                        nc.tensor.matmul(
                            ps[:],
                            outT[b][:, t_sl],
                            wout_sb[:, nt * 512 : (nt + 1) * 512],
                            start=True,
                            stop=True,
                        )
                        ob = ost.tile([128, 512], BF16, tag="o")
                        nc.vector.tensor_copy(ob[:], ps[:])
                        nc.sync.dma_start(
                            out_d.ap()[
                                b * SQ + tt * 128 : b * SQ + (tt + 1) * 128,
                                nt * 512 : (nt + 1) * 512,
                            ],
                            ob[:],
                        )

            for b in range(B):
                project(kt_sb[b], wk_sb, xkvt, b * SKV)
                project(vt_sb[b], wv_sb, xkvt, b * SKV)
                build_vnat(b)
                project(qt_sb[b], wq_sb, xqt, b * SQ)
                attention(b)
                outproj(b)

    nc.compile()
    return nc


def make_in_maps(x_q, x_kv, W_qkv, W_out):
    x_q = np.asarray(x_q, dtype=np.float32)
    x_kv = np.asarray(x_kv, dtype=np.float32)
    W_qkv = np.asarray(W_qkv, dtype=np.float32)
    W_out = np.asarray(W_out, dtype=np.float32)

    xqt = np.ascontiguousarray(x_q.reshape(TOK, DIM).T).astype(BF)
    xkvt = np.ascontiguousarray(x_kv.reshape(TOK, DIM).T).astype(BF)

    in_maps = []
    for c in range(N_CORES):
        cs = slice(c * HD, (c + 1) * HD)
        in_maps.append(
            {
                "xqt": xqt,
                "xkvt": xkvt,
                "wq": np.ascontiguousarray(W_qkv[:, cs]).astype(BF),
                "wk": np.ascontiguousarray(W_qkv[:, 1024:][:, cs]).astype(BF),
                "wv": np.ascontiguousarray(W_qkv[:, 2048:][:, cs]).astype(BF),
                "wout": np.ascontiguousarray(W_out[cs, :]).astype(BF),
            }
        )
    return in_maps


def combine(partials, b_out):
    """Sum the 8 per-core partial projections and add the bias."""
    acc = np.zeros((TOK, DIM), dtype=np.float32)
    for p in partials:
        acc += np.asarray(p, dtype=np.float32)
    acc += np.asarray(b_out, dtype=np.float32)
    return acc.reshape(B, SQ, DIM)


_STATE = {}


def _get_nc():
    if "nc" not in _STATE:
        _STATE["nc"] = build()
    return _STATE["nc"]


def run(x_q, x_kv, W_qkv, W_out, b_out, trace=False):
    nc = _get_nc()
    in_maps = make_in_maps(x_q, x_kv, W_qkv, W_out)
    res = run_bass_kernel_spmd(nc, in_maps, list(range(N_CORES)), trace=trace)
    out = combine([r["out"] for r in res.results], b_out)
    return out, res


def kernel(x_q, x_kv, W_qkv, W_out, b_out):
    out, _ = run(x_q, x_kv, W_qkv, W_out, b_out, trace=False)
    return out


# revision 9
# speedup vs baseline: 1.1780x; 1.1571x over previous
"""Multi-head attention (b=2, sq=skv=2048, dim=1024, 16 heads x 64) on 8 TRN2
NeuronCores.

Sharding: 2 heads per core (head-parallel across batch*heads), with the
matching tensor-parallel column slice of W_qkv and row slice of W_out.  Each
core computes a partial output projection over its 128 head-dims; the
all-reduce of the 8 partials (+ bias) happens on the host during unshard.

Per-core kernel (bf16 compute, fp32 PSUM accumulation):
  phase 1: qT/kT/vT = W.T @ x.T   ([128 = 2 heads x 64 dims, tokens]); v is
           additionally PE-transposed to natural [token, dim] layout with a
           ones column appended (denominator trick).
  phase 2: per (batch, q-tile, k-tile): scoresT for both heads ([k-tokens, q])
           in one 2-bank PSUM group; one exp ACTIVATE over the group (scale
           1/8 fused, no max subtraction -- scores range +-10); PV matmuls
           accumulate [v | 1].T @ expT over the 16 k-tiles giving unnormalized
           outT plus the softmax denominator in row 64.  The accumulator is
           copied to SBUF immediately (releasing PSUM); normalization
           (reciprocal + PE outer-product broadcast + multiply) happens off
           the critical path.
  phase 3: partial out = outT.T @ W_out_rows, DMA'd straight from PSUM to
           DRAM as fp32 [tokens, 1024].
"""

import os
import sys

for _p in ("/opt/trn_rl_repo", "/root/.axon_site/_ro/trn_rl_repo"):
    if os.path.isdir(_p) and _p not in sys.path:
        sys.path.append(_p)

import ml_dtypes
import numpy as np

import concourse.bass as bass  # noqa: F401
import concourse.tile as tile
from concourse import bacc, mybir
from concourse.bass_utils import run_bass_kernel_spmd
from concourse.masks import make_identity

B, SQ, SKV, DIM = 2, 2048, 2048, 1024
HEADS, DH = 16, 64
N_CORES = 8
HPC = HEADS // N_CORES  # heads per core = 2
HD = HPC * DH  # 128 head-dim rows per core
TOK = B * SQ  # 4096
KO = DIM // 128  # 8 contraction chunks of 128
SCALE = DH**-0.5

BF16 = mybir.dt.bfloat16
F32 = mybir.dt.float32

PCHUNK = 1024  # token chunk in projections (2KB dram lines)
QTILE = 512  # q tile in attention
KTILE = 128  # k tile (scores psum partition dim)
NKT = SKV // KTILE  # 16
NQT = SQ // QTILE  # 4

BF = ml_dtypes.bfloat16
Exp = mybir.ActivationFunctionType.Exp


def build():
    nc = bacc.Bacc(
        "TRN2", target_bir_lowering=False, debug=False, num_devices=N_CORES
    )

    xqt_d = nc.dram_tensor("xqt", [DIM, TOK], BF16, kind="ExternalInput")
    xkvt_d = nc.dram_tensor("xkvt", [DIM, TOK], BF16, kind="ExternalInput")
    wq_d = nc.dram_tensor("wq", [DIM, HD], BF16, kind="ExternalInput")
    wk_d = nc.dram_tensor("wk", [DIM, HD], BF16, kind="ExternalInput")
    wv_d = nc.dram_tensor("wv", [DIM, HD], BF16, kind="ExternalInput")
    wout_d = nc.dram_tensor("wout", [HD, DIM], BF16, kind="ExternalInput")
    out_d = nc.dram_tensor("out", [TOK, DIM], BF16, kind="ExternalOutput")

    xqt = xqt_d.ap().rearrange("(ko p) t -> p ko t", p=128)
    xkvt = xkvt_d.ap().rearrange("(ko p) t -> p ko t", p=128)

    with tile.TileContext(nc) as tc:
        with (
            tc.tile_pool(name="persist", bufs=1) as persist,
            tc.tile_pool(name="xin", bufs=3) as xin,
            tc.tile_pool(name="exps", bufs=6) as exps,
            tc.tile_pool(name="ost", bufs=3) as ost,
            tc.tile_pool(name="spsum", bufs=2, space="PSUM") as spsum,
            tc.tile_pool(name="accp", bufs=2, space="PSUM") as accp,
            tc.tile_pool(name="miscp", bufs=2, space="PSUM") as miscp,
            tc.tile_pool(name="drp", bufs=2, space="DRAM") as drp,
        ):
            # --- weights / constants ---
            wq_sb = persist.tile([128, KO, HD], BF16, tag="wq")
            nc.sync.dma_start(wq_sb[:], wq_d.ap().rearrange("(ko p) m -> p ko m", p=128))
            wk_sb = persist.tile([128, KO, HD], BF16, tag="wk")
            nc.sync.dma_start(wk_sb[:], wk_d.ap().rearrange("(ko p) m -> p ko m", p=128))
            wv_sb = persist.tile([128, KO, HD], BF16, tag="wv")
            nc.sync.dma_start(wv_sb[:], wv_d.ap().rearrange("(ko p) m -> p ko m", p=128))
            wout_sb = persist.tile([HD, DIM], BF16, tag="wout")
            nc.sync.dma_start(wout_sb[:], wout_d.ap())

            ident = persist.tile([128, DH], BF16, tag="ident")
            make_identity(nc, ident[0:DH, :])
            make_identity(nc, ident[DH : 2 * DH, :])
            ones_f32 = persist.tile([1, DH], F32, tag="ones")
            nc.vector.memset(ones_f32[:], 1.0)

            qt_sb, kt_sb, vt_sb, vnat, outT, usb = {}, {}, {}, {}, {}, {}
            for b in range(B):
                qt_sb[b] = persist.tile([HD, SQ], BF16, tag=f"qt{b}", name=f"qt{b}")
                kt_sb[b] = persist.tile([HD, SKV], BF16, tag=f"kt{b}", name=f"kt{b}")
                vt_sb[b] = persist.tile([HD, SKV], BF16, tag=f"vt{b}", name=f"vt{b}")
                vnat[b] = persist.tile(
                    [128, HPC, NKT, DH + 1], BF16, tag=f"vn{b}", name=f"vn{b}"
                )
                outT[b] = persist.tile([HD, SQ], BF16, tag=f"ot{b}", name=f"ot{b}")
                # unnormalized outT + denominators, unit index = qt*HPC + h
                usb[b] = persist.tile(
                    [DH + 1, NQT * HPC, QTILE], F32, tag=f"us{b}", name=f"us{b}"
                )
                nc.vector.memset(vnat[b][:, :, :, DH], 1.0)

            def project(dst, w_sb, x_ap, tok0):
                """dst[:, t] = sum_D w[D, :] x[D, tok0 + t] for SQ tokens."""
                for tt in range(SQ // PCHUNK):
                    xt = xin.tile([128, KO, PCHUNK], BF16, tag="x")
                    nc.sync.dma_start(
                        xt[:],
                        x_ap[:, :, tok0 + tt * PCHUNK : tok0 + (tt + 1) * PCHUNK],
                    )
                    for sub in range(PCHUNK // 512):
                        ps = miscp.tile([128, 512], F32, tag="m", name="projp")
                        for ko in range(KO):
                            nc.tensor.matmul(
                                ps[:],
                                w_sb[:, ko, :],
                                xt[:, ko, sub * 512 : (sub + 1) * 512],
                                start=(ko == 0),
                                stop=(ko == KO - 1),
                            )
                        t0 = tt * PCHUNK + sub * 512
                        nc.vector.tensor_copy(dst[:, t0 : t0 + 512], ps[:])

            def build_vnat(b):
                for jg in range(NKT // 4):
                    for h in range(HPC):
                        tp = miscp.tile([128, 4, DH], BF16, tag="m", name="vtp")
                        for i in range(4):
                            j = jg * 4 + i
                            nc.tensor.transpose(
                                tp[:, i, :],
                                vt_sb[b][
                                    h * DH : (h + 1) * DH,
                                    j * KTILE : (j + 1) * KTILE,
                                ],
                                ident[h * DH : (h + 1) * DH, :],
                            )
                        nc.vector.tensor_copy(
                            vnat[b][:, h, jg * 4 : (jg + 1) * 4, 0:DH], tp[:]
                        )

            F32R = mybir.dt.float32r

            def norm_flush(b, u0, nu):
                """Normalize units u0..u0+nu-1 of usb[b] into outT[b].

                Batches the reciprocal: denominator rows are bounced through
                DRAM to repack [1, nu, QTILE] -> [128, nu*QTILE/128] so the
                DVE reciprocal runs wide, then bounced back and broadcast to
                64 partitions with an f32r PE outer product.
                """
                nel = nu * QTILE
                d1 = drp.tile([1, nu, QTILE], F32, tag="d1", name="d1")
                nc.sync.dma_start(d1[:], usb[b][DH : DH + 1, u0 : u0 + nu, :])
                dpk = ost.tile([128, nel // 128], F32, tag="dp", name="dpk")
                nc.sync.dma_start(
                    dpk[:],
                    d1[:]
                    .rearrange("a b c -> (a b c)")
                    .rearrange("(p f) -> p f", p=128),
                )
                rpk = ost.tile([128, nel // 128], F32, tag="rp", name="rpk")
                nc.vector.reciprocal(rpk[:], dpk[:])
                d2 = drp.tile([1, nu, QTILE], F32, tag="d2", name="d2")
                nc.sync.dma_start(
                    d2[:]
                    .rearrange("a b c -> (a b c)")
                    .rearrange("(p f) -> p f", p=128),
                    rpk[:],
                )
                rst = ost.tile([1, nu, QTILE], F32, tag="rs", name="rst")
                nc.sync.dma_start(rst[:], d2[:])
                for i in range(nu):
                    g = u0 + i
                    qt, h = divmod(g, HPC)
                    bc = miscp.tile([DH, QTILE], F32, tag="m", name="bc")
                    nc.tensor.matmul(
                        bc[:],
                        ones_f32[:].bitcast(F32R),
                        rst[0:1, i, :].bitcast(F32R),
                        start=True,
                        stop=True,
                    )
                    nc.vector.tensor_mul(
                        outT[b][h * DH : (h + 1) * DH, qt * QTILE : (qt + 1) * QTILE],
                        usb[b][0:DH, g, :],
                        bc[:],
                    )

            def attention(b):
                for qt in range(NQT):
                    q_sl = slice(qt * QTILE, (qt + 1) * QTILE)
                    accs = [
                        accp.tile([128, QTILE], F32, tag="acc", name="acc")
                        for _ in range(HPC)
                    ]
                    for j in range(NKT):
                        k_sl = slice(j * KTILE, (j + 1) * KTILE)
                        sp = spsum.tile([128, HPC, QTILE], F32, tag="s")
                        for h in range(HPC):
                            h_sl = slice(h * DH, (h + 1) * DH)
                            nc.tensor.matmul(
                                sp[:, h, :],
                                kt_sb[b][h_sl, k_sl],
                                qt_sb[b][h_sl, q_sl],
                                start=True,
                                stop=True,
                            )
                        ex = exps.tile([128, HPC, QTILE], BF16, tag="e")
                        nc.scalar.activation(ex[:], sp[:], Exp, scale=SCALE)
                        for h in range(HPC):
                            nc.tensor.matmul(
                                accs[h][0 : DH + 1, :],
                                vnat[b][:, h, j, :],
                                ex[:, h, :],
                                start=(j == 0),
                                stop=(j == NKT - 1),
                            )
                    for h in range(HPC):
                        # free the PSUM accumulator immediately; normalization
                        # happens later in norm_flush
                        nc.vector.tensor_copy(
                            usb[b][:, qt * HPC + h, :], accs[h][0 : DH + 1, :]
                        )
                    if qt == 1:
                        norm_flush(b, 0, 2 * HPC)
                if NQT > 2:
                    norm_flush(b, 2 * HPC, (NQT - 2) * HPC)

            def outproj(b):
                for tt in range(SQ // 128):
                    t_sl = slice(tt * 128, (tt + 1) * 128)
                    for nt in range(DIM // 512):
                        ps = miscp.tile([128, 512], F32, tag="m", name="projo")
                        nc.tensor.matmul(
                            ps[:],
                            outT[b][:, t_sl],
                            wout_sb[:, nt * 512 : (nt + 1) * 512],
                            start=True,
                            stop=True,
                        )
                        ob = ost.tile([128, 512], BF16, tag="o")
                        nc.vector.tensor_copy(ob[:], ps[:])
                        nc.sync.dma_start(
                            out_d.ap()[
                                b * SQ + tt * 128 : b * SQ + (tt + 1) * 128,
                                nt * 512 : (nt + 1) * 512,
                            ],
                            ob[:],
                        )

            for b in range(B):
                project(kt_sb[b], wk_sb, xkvt, b * SKV)
                project(vt_sb[b], wv_sb, xkvt, b * SKV)
                build_vnat(b)
                project(qt_sb[b], wq_sb, xqt, b * SQ)
                attention(b)
                outproj(b)

    nc.compile()
    return nc


def make_in_maps(x_q, x_kv, W_qkv, W_out):
    x_q = np.asarray(x_q, dtype=np.float32)
    x_kv = np.asarray(x_kv, dtype=np.float32)
    W_qkv = np.asarray(W_qkv, dtype=np.float32)
    W_out = np.asarray(W_out, dtype=np.float32)

    xqt = np.ascontiguousarray(x_q.reshape(TOK, DIM).T).astype(BF)
    xkvt = np.ascontiguousarray(x_kv.reshape(TOK, DIM).T).astype(BF)

    in_maps = []
    for c in range(N_CORES):
        cs = slice(c * HD, (c + 1) * HD)
        in_maps.append(
            {
                "xqt": xqt,
                "xkvt": xkvt,
                "wq": np.ascontiguousarray(W_qkv[:, cs]).astype(BF),
                "wk": np.ascontiguousarray(W_qkv[:, 1024:][:, cs]).astype(BF),
                "wv": np.ascontiguousarray(W_qkv[:, 2048:][:, cs]).astype(BF),
                "wout": np.ascontiguousarray(W_out[cs, :]).astype(BF),
            }
        )
    return in_maps


def combine(partials, b_out):
    """Sum the 8 per-core partial projections and add the bias."""
    acc = np.zeros((TOK, DIM), dtype=np.float32)
    for p in partials:
        acc += np.asarray(p, dtype=np.float32)
    acc += np.asarray(b_out, dtype=np.float32)
    return acc.reshape(B, SQ, DIM)


_STATE = {}


def _get_nc():
    if "nc" not in _STATE:
        _STATE["nc"] = build()
    return _STATE["nc"]


def run(x_q, x_kv, W_qkv, W_out, b_out, trace=False):
    nc = _get_nc()
    in_maps = make_in_maps(x_q, x_kv, W_qkv, W_out)
    res = run_bass_kernel_spmd(nc, in_maps, list(range(N_CORES)), trace=trace)
    out = combine([r["out"] for r in res.results], b_out)
    return out, res


def kernel(x_q, x_kv, W_qkv, W_out, b_out):
    out, _ = run(x_q, x_kv, W_qkv, W_out, b_out, trace=False)
    return out


# revision 12
# speedup vs baseline: 1.4163x; 1.2023x over previous
"""Multi-head attention (b=2, sq=skv=2048, dim=1024, 16 heads x 64) on 8 TRN2
NeuronCores.

Sharding: 2 heads per core (head-parallel across batch*heads), with the
matching tensor-parallel column slice of W_qkv and row slice of W_out.  Each
core computes a partial output projection over its 128 head-dims; the
all-reduce of the 8 partials (+ bias) happens on the host during unshard.

Per-core kernel (bf16 compute, fp32 PSUM accumulation):
  phase 1: qT/kT/vT = W.T @ x.T   ([128 = 2 heads x 64 dims, tokens]); v is
           additionally PE-transposed to natural [token, dim] layout with a
           ones column appended (denominator trick).
  phase 2: per (batch, q-tile, k-tile): scoresT for both heads ([k-tokens, q])
           in one 2-bank PSUM group; one exp ACTIVATE over the group (scale
           1/8 fused, no max subtraction -- scores range +-10); PV matmuls
           accumulate [v | 1].T @ expT over the 16 k-tiles giving unnormalized
           outT plus the softmax denominator in row 64.  The accumulator is
           copied to SBUF immediately (releasing PSUM); normalization
           (reciprocal + PE outer-product broadcast + multiply) happens off
           the critical path.
  phase 3: partial out = outT.T @ W_out_rows, DMA'd straight from PSUM to
           DRAM as fp32 [tokens, 1024].
"""

import os
import sys

for _p in ("/opt/trn_rl_repo", "/root/.axon_site/_ro/trn_rl_repo"):
    if os.path.isdir(_p) and _p not in sys.path:
        sys.path.append(_p)

import ml_dtypes
import numpy as np

import concourse.bass as bass  # noqa: F401
import concourse.tile as tile
from concourse import bacc, mybir
from concourse.bass_utils import run_bass_kernel_spmd
from concourse.masks import make_identity

B, SQ, SKV, DIM = 2, 2048, 2048, 1024
HEADS, DH = 16, 64
N_CORES = 8
HPC = HEADS // N_CORES  # heads per core = 2
HD = HPC * DH  # 128 head-dim rows per core
TOK = B * SQ  # 4096
KO = DIM // 128  # 8 contraction chunks of 128
SCALE = DH**-0.5

BF16 = mybir.dt.bfloat16
F32 = mybir.dt.float32

PCHUNK = 1024  # token chunk in projections (2KB dram lines)
QTILE = 512  # q tile in attention
KTILE = 128  # k tile (scores psum partition dim)
NKT = SKV // KTILE  # 16
NQT = SQ // QTILE  # 4

BF = ml_dtypes.bfloat16
Exp = mybir.ActivationFunctionType.Exp


def build():
    nc = bacc.Bacc(
        "TRN2", target_bir_lowering=False, debug=False, num_devices=N_CORES
    )

    xqt_d = nc.dram_tensor("xqt", [DIM, TOK], BF16, kind="ExternalInput")
    xkvt_d = nc.dram_tensor("xkvt", [DIM, TOK], BF16, kind="ExternalInput")
    wq_d = nc.dram_tensor("wq", [DIM, HD], BF16, kind="ExternalInput")
    wk_d = nc.dram_tensor("wk", [DIM, HD], BF16, kind="ExternalInput")
    wv_d = nc.dram_tensor("wv", [DIM, HD], BF16, kind="ExternalInput")
    wout_d = nc.dram_tensor("wout", [HD, DIM], BF16, kind="ExternalInput")
    out_d = nc.dram_tensor("out", [TOK, DIM], BF16, kind="ExternalOutput")

    xqt = xqt_d.ap().rearrange("(ko p) t -> p ko t", p=128)
    xkvt = xkvt_d.ap().rearrange("(ko p) t -> p ko t", p=128)

    with tile.TileContext(nc) as tc:
        with (
            tc.tile_pool(name="persist", bufs=1) as persist,
            tc.tile_pool(name="xin", bufs=3) as xin,
            tc.tile_pool(name="exps", bufs=6) as exps,
            tc.tile_pool(name="ost", bufs=3) as ost,
            tc.tile_pool(name="spsum", bufs=2, space="PSUM") as spsum,
            tc.tile_pool(name="accp", bufs=2, space="PSUM") as accp,
            tc.tile_pool(name="miscp", bufs=2, space="PSUM") as miscp,
            tc.tile_pool(name="drp", bufs=2, space="DRAM") as drp,
        ):
            # --- weights / constants ---
            wq_sb = persist.tile([128, KO, HD], BF16, tag="wq")
            nc.sync.dma_start(wq_sb[:], wq_d.ap().rearrange("(ko p) m -> p ko m", p=128))
            wk_sb = persist.tile([128, KO, HD], BF16, tag="wk")
            nc.sync.dma_start(wk_sb[:], wk_d.ap().rearrange("(ko p) m -> p ko m", p=128))
            wv_sb = persist.tile([128, KO, HD], BF16, tag="wv")
            nc.sync.dma_start(wv_sb[:], wv_d.ap().rearrange("(ko p) m -> p ko m", p=128))
            wout_sb = persist.tile([HD, DIM], BF16, tag="wout")
            nc.sync.dma_start(wout_sb[:], wout_d.ap())

            ident = persist.tile([128, DH], BF16, tag="ident")
            make_identity(nc, ident[0:DH, :])
            make_identity(nc, ident[DH : 2 * DH, :])
            ones_f32 = persist.tile([1, DH], F32, tag="ones")
            nc.vector.memset(ones_f32[:], 1.0)

            qt_sb, kt_sb, vt_sb, vnat, outT, usb = {}, {}, {}, {}, {}, {}
            for b in range(B):
                qt_sb[b] = persist.tile([HD, SQ], BF16, tag=f"qt{b}", name=f"qt{b}")
                kt_sb[b] = persist.tile([HD, SKV], BF16, tag=f"kt{b}", name=f"kt{b}")
                vt_sb[b] = persist.tile([HD, SKV], BF16, tag=f"vt{b}", name=f"vt{b}")
                vnat[b] = persist.tile(
                    [128, HPC, NKT, DH + 1], BF16, tag=f"vn{b}", name=f"vn{b}"
                )
                outT[b] = persist.tile([HD, SQ], BF16, tag=f"ot{b}", name=f"ot{b}")
                # unnormalized outT + denominators, unit index = qt*HPC + h
                usb[b] = persist.tile(
                    [DH + 1, NQT * HPC, QTILE], F32, tag=f"us{b}", name=f"us{b}"
                )
                nc.vector.memset(vnat[b][:, :, :, DH], 1.0)

            def proj_chunk(dst, w_sb, x_ap, tok0, tt):
                """Project one PCHUNK of tokens into dst[:, tt*PCHUNK...]."""
                xt = xin.tile([128, KO, PCHUNK], BF16, tag="x")
                nc.sync.dma_start(
                    xt[:],
                    x_ap[:, :, tok0 + tt * PCHUNK : tok0 + (tt + 1) * PCHUNK],
                )
                for sub in range(PCHUNK // 512):
                    ps = miscp.tile([128, 512], F32, tag="m", name="projp")
                    for ko in range(KO):
                        nc.tensor.matmul(
                            ps[:],
                            w_sb[:, ko, :],
                            xt[:, ko, sub * 512 : (sub + 1) * 512],
                            start=(ko == 0),
                            stop=(ko == KO - 1),
                        )
                    t0 = tt * PCHUNK + sub * 512
                    nc.vector.tensor_copy(dst[:, t0 : t0 + 512], ps[:])

            def vnat_group(b, jg):
                """PE-transpose k-tiles 4jg..4jg+3 of vT into natural layout."""
                for h in range(HPC):
                    tp = miscp.tile([128, 4, DH], BF16, tag="m", name="vtp")
                    for i in range(4):
                        j = jg * 4 + i
                        nc.tensor.transpose(
                            tp[:, i, :],
                            vt_sb[b][
                                h * DH : (h + 1) * DH,
                                j * KTILE : (j + 1) * KTILE,
                            ],
                            ident[h * DH : (h + 1) * DH, :],
                        )
                    nc.vector.tensor_copy(
                        vnat[b][:, h, jg * 4 : (jg + 1) * 4, 0:DH], tp[:]
                    )

            F32R = mybir.dt.float32r

            def norm_flush(b, u0, nu):
                """Normalize units u0..u0+nu-1 of usb[b] into outT[b].

                Batches the reciprocal: denominator rows are bounced through
                DRAM to repack [1, nu, QTILE] -> [128, nu*QTILE/128] so the
                DVE reciprocal runs wide, then bounced back and broadcast to
                64 partitions with an f32r PE outer product.
                """
                nel = nu * QTILE
                d1 = drp.tile([1, nu, QTILE], F32, tag="d1", name="d1")
                nc.gpsimd.dma_start(d1[:], usb[b][DH : DH + 1, u0 : u0 + nu, :])
                dpk = ost.tile([128, nel // 128], F32, tag="dp", name="dpk")
                nc.gpsimd.dma_start(
                    dpk[:],
                    d1[:]
                    .rearrange("a b c -> (a b c)")
                    .rearrange("(p f) -> p f", p=128),
                )
                rpk = ost.tile([128, nel // 128], F32, tag="rp", name="rpk")
                nc.vector.reciprocal(rpk[:], dpk[:])
                d2 = drp.tile([1, nu, QTILE], F32, tag="d2", name="d2")
                nc.gpsimd.dma_start(
                    d2[:]
                    .rearrange("a b c -> (a b c)")
                    .rearrange("(p f) -> p f", p=128),
                    rpk[:],
                )
                rst = ost.tile([1, nu, QTILE], F32, tag="rs", name="rst")
                nc.gpsimd.dma_start(rst[:], d2[:])
                for i in range(nu):
                    g = u0 + i
                    qt, h = divmod(g, HPC)
                    bc = miscp.tile([DH, QTILE], F32, tag="m", name="bc")
                    nc.tensor.matmul(
                        bc[:],
                        ones_f32[:].bitcast(F32R),
                        rst[0:1, i, :].bitcast(F32R),
                        start=True,
                        stop=True,
                    )
                    nc.vector.tensor_mul(
                        outT[b][h * DH : (h + 1) * DH, qt * QTILE : (qt + 1) * QTILE],
                        usb[b][0:DH, g, :],
                        bc[:],
                    )

            def attn_qt(b, qt):
                """One q-tile of attention for both heads of batch b."""
                q_sl = slice(qt * QTILE, (qt + 1) * QTILE)
                accs = [
                    accp.tile([128, QTILE], F32, tag="acc", name="acc")
                    for _ in range(HPC)
                ]
                for j in range(NKT):
                    k_sl = slice(j * KTILE, (j + 1) * KTILE)
                    sp = spsum.tile([128, HPC, QTILE], F32, tag="s")
                    for h in range(HPC):
                        h_sl = slice(h * DH, (h + 1) * DH)
                        nc.tensor.matmul(
                            sp[:, h, :],
                            kt_sb[b][h_sl, k_sl],
                            qt_sb[b][h_sl, q_sl],
                            start=True,
                            stop=True,
                        )
                    ex = exps.tile([128, HPC, QTILE], BF16, tag="e")
                    nc.scalar.activation(ex[:], sp[:], Exp, scale=SCALE)
                    for h in range(HPC):
                        nc.tensor.matmul(
                            accs[h][0 : DH + 1, :],
                            vnat[b][:, h, j, :],
                            ex[:, h, :],
                            start=(j == 0),
                            stop=(j == NKT - 1),
                        )
                for h in range(HPC):
                    # free the PSUM accumulator immediately; normalization
                    # happens later in norm_flush
                    nc.vector.tensor_copy(
                        usb[b][:, qt * HPC + h, :], accs[h][0 : DH + 1, :]
                    )

            def outproj(b, tt0, tt1):
                for tt in range(tt0, tt1):
                    t_sl = slice(tt * 128, (tt + 1) * 128)
                    for nt in range(DIM // 512):
                        ps = miscp.tile([128, 512], F32, tag="m", name="projo")
                        nc.tensor.matmul(
                            ps[:],
                            outT[b][:, t_sl],
                            wout_sb[:, nt * 512 : (nt + 1) * 512],
                            start=True,
                            stop=True,
                        )
                        ob = ost.tile([128, 512], BF16, tag="o")
                        nc.vector.tensor_copy(ob[:], ps[:])
                        nc.gpsimd.dma_start(
                            out_d.ap()[
                                b * SQ + tt * 128 : b * SQ + (tt + 1) * 128,
                                nt * 512 : (nt + 1) * 512,
                            ],
                            ob[:],
                        )

            def qkv_pieces(b):
                """Generator of fine-grained projection emission steps."""
                yield lambda: proj_chunk(kt_sb[b], wk_sb, xkvt, b * SKV, 0)
                yield lambda: proj_chunk(vt_sb[b], wv_sb, xkvt, b * SKV, 0)
                yield lambda: (vnat_group(b, 0), vnat_group(b, 1))
                yield lambda: proj_chunk(qt_sb[b], wq_sb, xqt, b * SQ, 0)
                yield lambda: proj_chunk(kt_sb[b], wk_sb, xkvt, b * SKV, 1)
                yield lambda: proj_chunk(vt_sb[b], wv_sb, xkvt, b * SKV, 1)
                yield lambda: (vnat_group(b, 2), vnat_group(b, 3))
                yield lambda: proj_chunk(qt_sb[b], wq_sb, xqt, b * SQ, 1)

            # --- emission schedule: fine-grained interleave so the scheduler
            # always has dep-free PE work to fill ACT-bound attention gaps ---
            for piece in qkv_pieces(0):
                piece()

            nxt = qkv_pieces(1)

            def emit_next(n):
                for _ in range(n):
                    p = next(nxt, None)
                    if p is not None:
                        p()

            attn_qt(0, 0)
            emit_next(2)  # kt(b1) c0, vt(b1) c0
            attn_qt(0, 1)
            emit_next(2)  # vnat(b1) 0-1, qt(b1) c0
            norm_flush(0, 0, 2 * HPC)
            attn_qt(0, 2)
            emit_next(2)  # kt(b1) c1, vt(b1) c1
            outproj(0, 0, 8)
            attn_qt(0, 3)
            emit_next(2)  # vnat(b1) 2-3, qt(b1) c1
            norm_flush(0, 2 * HPC, 2 * HPC)

            attn_qt(1, 0)
            outproj(0, 8, 12)
            attn_qt(1, 1)
            outproj(0, 12, 16)
            norm_flush(1, 0, 2 * HPC)
            attn_qt(1, 2)
            outproj(1, 0, 8)
            attn_qt(1, 3)
            norm_flush(1, 2 * HPC, 2 * HPC)
            outproj(1, 8, 16)

    nc.compile()
    return nc


def make_in_maps(x_q, x_kv, W_qkv, W_out):
    x_q = np.asarray(x_q, dtype=np.float32)
    x_kv = np.asarray(x_kv, dtype=np.float32)
    W_qkv = np.asarray(W_qkv, dtype=np.float32)
    W_out = np.asarray(W_out, dtype=np.float32)

    xqt = np.ascontiguousarray(x_q.reshape(TOK, DIM).T).astype(BF)
    xkvt = np.ascontiguousarray(x_kv.reshape(TOK, DIM).T).astype(BF)

    in_maps = []
    for c in range(N_CORES):
        cs = slice(c * HD, (c + 1) * HD)
        in_maps.append(
            {
                "xqt": xqt,
                "xkvt": xkvt,
                "wq": np.ascontiguousarray(W_qkv[:, cs]).astype(BF),
                "wk": np.ascontiguousarray(W_qkv[:, 1024:][:, cs]).astype(BF),
                "wv": np.ascontiguousarray(W_qkv[:, 2048:][:, cs]).astype(BF),
                "wout": np.ascontiguousarray(W_out[cs, :]).astype(BF),
            }
        )
    return in_maps


def combine(partials, b_out):
    """Sum the 8 per-core partial projections and add the bias."""
    acc = np.zeros((TOK, DIM), dtype=np.float32)
    for p in partials:
        acc += np.asarray(p, dtype=np.float32)
    acc += np.asarray(b_out, dtype=np.float32)
    return acc.reshape(B, SQ, DIM)


_STATE = {}


def _get_nc():
    if "nc" not in _STATE:
        _STATE["nc"] = build()
    return _STATE["nc"]


def run(x_q, x_kv, W_qkv, W_out, b_out, trace=False):
    nc = _get_nc()
    in_maps = make_in_maps(x_q, x_kv, W_qkv, W_out)
    res = run_bass_kernel_spmd(nc, in_maps, list(range(N_CORES)), trace=trace)
    out = combine([r["out"] for r in res.results], b_out)
    return out, res


def kernel(x_q, x_kv, W_qkv, W_out, b_out):
    out, _ = run(x_q, x_kv, W_qkv, W_out, b_out, trace=False)
    return out


# revision 14
# speedup vs baseline: 1.5255x; 1.0771x over previous
"""Multi-head attention (b=2, sq=skv=2048, dim=1024, 16 heads x 64) on 8 TRN2
NeuronCores.

Sharding: 2 heads per core (head-parallel across batch*heads), with the
matching tensor-parallel column slice of W_qkv and row slice of W_out.  Each
core computes a partial output projection over its 128 head-dims; the
all-reduce of the 8 partials (+ bias) happens on the host during unshard.

Per-core kernel (bf16 compute, fp32 PSUM accumulation):
  phase 1: qT/kT/vT = W.T @ x.T   ([128 = 2 heads x 64 dims, tokens]); v is
           additionally PE-transposed to natural [token, dim] layout with a
           ones column appended (denominator trick).
  phase 2: per (batch, q-tile, k-tile): scoresT for both heads ([k-tokens, q])
           in one 2-bank PSUM group; one exp ACTIVATE over the group (scale
           1/8 fused, no max subtraction -- scores range +-10); PV matmuls
           accumulate [v | 1].T @ expT over the 16 k-tiles giving unnormalized
           outT plus the softmax denominator in row 64.  The accumulator is
           copied to SBUF immediately (releasing PSUM); normalization
           (reciprocal + PE outer-product broadcast + multiply) happens off
           the critical path.
  phase 3: partial out = outT.T @ W_out_rows, DMA'd straight from PSUM to
           DRAM as fp32 [tokens, 1024].
"""

import os
import sys

for _p in ("/opt/trn_rl_repo", "/root/.axon_site/_ro/trn_rl_repo"):
    if os.path.isdir(_p) and _p not in sys.path:
        sys.path.append(_p)

import ml_dtypes
import numpy as np

import concourse.bass as bass  # noqa: F401
import concourse.tile as tile
from concourse import bacc, mybir
from concourse.bass_utils import run_bass_kernel_spmd
from concourse.masks import make_identity

B, SQ, SKV, DIM = 2, 2048, 2048, 1024
HEADS, DH = 16, 64
N_CORES = 8
HPC = HEADS // N_CORES  # heads per core = 2
HD = HPC * DH  # 128 head-dim rows per core
TOK = B * SQ  # 4096
KO = DIM // 128  # 8 contraction chunks of 128
SCALE = DH**-0.5

BF16 = mybir.dt.bfloat16
F32 = mybir.dt.float32

PCHUNK = 1024  # token chunk in projections (2KB dram lines)
QTILE = 512  # q tile in attention
KTILE = 128  # k tile (scores psum partition dim)
NKT = SKV // KTILE  # 16
NQT = SQ // QTILE  # 4

BF = ml_dtypes.bfloat16
Exp = mybir.ActivationFunctionType.Exp


def build():
    nc = bacc.Bacc(
        "TRN2", target_bir_lowering=False, debug=False, num_devices=N_CORES
    )

    xqt_d = nc.dram_tensor("xqt", [DIM, TOK], BF16, kind="ExternalInput")
    xkvt_d = nc.dram_tensor("xkvt", [DIM, TOK], BF16, kind="ExternalInput")
    wq_d = nc.dram_tensor("wq", [DIM, HD], BF16, kind="ExternalInput")
    wk_d = nc.dram_tensor("wk", [DIM, HD], BF16, kind="ExternalInput")
    wv_d = nc.dram_tensor("wv", [DIM, HD], BF16, kind="ExternalInput")
    wout_d = nc.dram_tensor("wout", [HD, DIM], BF16, kind="ExternalInput")
    out_d = nc.dram_tensor("out", [TOK, DIM], BF16, kind="ExternalOutput")

    xqt = xqt_d.ap().rearrange("(ko p) t -> p ko t", p=128)
    xkvt = xkvt_d.ap().rearrange("(ko p) t -> p ko t", p=128)

    with tile.TileContext(nc) as tc:
        with (
            tc.tile_pool(name="persist", bufs=1) as persist,
            tc.tile_pool(name="xin", bufs=3) as xin,
            tc.tile_pool(name="exps", bufs=6) as exps,
            tc.tile_pool(name="ost", bufs=3) as ost,
            tc.tile_pool(name="spsum", bufs=2, space="PSUM") as spsum,
            tc.tile_pool(name="accp", bufs=2, space="PSUM") as accp,
            tc.tile_pool(name="miscp", bufs=2, space="PSUM") as miscp,
            tc.tile_pool(name="drp", bufs=2, space="DRAM") as drp,
        ):
            # --- weights / constants ---
            wq_sb = persist.tile([128, KO, HD], BF16, tag="wq")
            nc.sync.dma_start(wq_sb[:], wq_d.ap().rearrange("(ko p) m -> p ko m", p=128))
            wk_sb = persist.tile([128, KO, HD], BF16, tag="wk")
            nc.sync.dma_start(wk_sb[:], wk_d.ap().rearrange("(ko p) m -> p ko m", p=128))
            wv_sb = persist.tile([128, KO, HD], BF16, tag="wv")
            nc.sync.dma_start(wv_sb[:], wv_d.ap().rearrange("(ko p) m -> p ko m", p=128))
            wout_sb = persist.tile([HD, DIM], BF16, tag="wout")
            nc.sync.dma_start(wout_sb[:], wout_d.ap())

            ident = persist.tile([128, DH], BF16, tag="ident")
            make_identity(nc, ident[0:DH, :])
            make_identity(nc, ident[DH : 2 * DH, :])
            ones_f32 = persist.tile([1, DH], F32, tag="ones")
            nc.vector.memset(ones_f32[:], 1.0)

            qt_sb, kt_sb, vt_sb, vnat, outT, usb = {}, {}, {}, {}, {}, {}
            for b in range(B):
                qt_sb[b] = persist.tile([HD, SQ], BF16, tag=f"qt{b}", name=f"qt{b}")
                kt_sb[b] = persist.tile([HD, SKV], BF16, tag=f"kt{b}", name=f"kt{b}")
                vt_sb[b] = persist.tile([HD, SKV], BF16, tag=f"vt{b}", name=f"vt{b}")
                vnat[b] = persist.tile(
                    [128, HPC, NKT, DH + 1], BF16, tag=f"vn{b}", name=f"vn{b}"
                )
                outT[b] = persist.tile([HD, SQ], BF16, tag=f"ot{b}", name=f"ot{b}")
                # unnormalized outT + denominators, unit index = qt*HPC + h
                usb[b] = persist.tile(
                    [DH + 1, NQT * HPC, QTILE], F32, tag=f"us{b}", name=f"us{b}"
                )
                nc.vector.memset(vnat[b][:, :, :, DH], 1.0)

            def _proj(dst, w_sb, xt, tt):
                for sub in range(PCHUNK // 512):
                    ps = miscp.tile([128, 512], F32, tag="m", name="projp")
                    for ko in range(KO):
                        nc.tensor.matmul(
                            ps[:],
                            w_sb[:, ko, :],
                            xt[:, ko, sub * 512 : (sub + 1) * 512],
                            start=(ko == 0),
                            stop=(ko == KO - 1),
                        )
                    t0 = tt * PCHUNK + sub * 512
                    nc.vector.tensor_copy(dst[:, t0 : t0 + 512], ps[:])

            def load_chunk(x_ap, tok0, tt):
                xt = xin.tile([128, KO, PCHUNK], BF16, tag="x")
                nc.sync.dma_start(
                    xt[:],
                    x_ap[:, :, tok0 + tt * PCHUNK : tok0 + (tt + 1) * PCHUNK],
                )
                return xt

            def proj_chunk(dst, w_sb, x_ap, tok0, tt):
                """Project one PCHUNK of tokens into dst[:, tt*PCHUNK...]."""
                _proj(dst, w_sb, load_chunk(x_ap, tok0, tt), tt)

            def vnat_group(b, jg):
                """PE-transpose k-tiles 4jg..4jg+3 of vT into natural layout."""
                for h in range(HPC):
                    tp = miscp.tile([128, 4, DH], BF16, tag="m", name="vtp")
                    for i in range(4):
                        j = jg * 4 + i
                        nc.tensor.transpose(
                            tp[:, i, :],
                            vt_sb[b][
                                h * DH : (h + 1) * DH,
                                j * KTILE : (j + 1) * KTILE,
                            ],
                            ident[h * DH : (h + 1) * DH, :],
                        )
                    nc.vector.tensor_copy(
                        vnat[b][:, h, jg * 4 : (jg + 1) * 4, 0:DH], tp[:]
                    )

            F32R = mybir.dt.float32r

            def norm_flush(b, u0, nu):
                """Normalize units u0..u0+nu-1 of usb[b] into outT[b].

                Batches the reciprocal: denominator rows are bounced through
                DRAM to repack [1, nu, QTILE] -> [128, nu*QTILE/128] so the
                DVE reciprocal runs wide, then bounced back and broadcast to
                64 partitions with an f32r PE outer product.
                """
                nel = nu * QTILE
                d1 = drp.tile([1, nu, QTILE], F32, tag="d1", name="d1")
                nc.gpsimd.dma_start(d1[:], usb[b][DH : DH + 1, u0 : u0 + nu, :])
                dpk = ost.tile([128, nel // 128], F32, tag="dp", name="dpk")
                nc.gpsimd.dma_start(
                    dpk[:],
                    d1[:]
                    .rearrange("a b c -> (a b c)")
                    .rearrange("(p f) -> p f", p=128),
                )
                rpk = ost.tile([128, nel // 128], F32, tag="rp", name="rpk")
                nc.vector.reciprocal(rpk[:], dpk[:])
                d2 = drp.tile([1, nu, QTILE], F32, tag="d2", name="d2")
                nc.gpsimd.dma_start(
                    d2[:]
                    .rearrange("a b c -> (a b c)")
                    .rearrange("(p f) -> p f", p=128),
                    rpk[:],
                )
                rst = ost.tile([1, nu, QTILE], F32, tag="rs", name="rst")
                nc.gpsimd.dma_start(rst[:], d2[:])
                for i in range(nu):
                    g = u0 + i
                    qt, h = divmod(g, HPC)
                    bc = miscp.tile([DH, QTILE], F32, tag="m", name="bc")
                    nc.tensor.matmul(
                        bc[:],
                        ones_f32[:].bitcast(F32R),
                        rst[0:1, i, :].bitcast(F32R),
                        start=True,
                        stop=True,
                    )
                    nc.vector.tensor_mul(
                        outT[b][h * DH : (h + 1) * DH, qt * QTILE : (qt + 1) * QTILE],
                        usb[b][0:DH, g, :],
                        bc[:],
                    )

            def attn_qt(b, qt):
                """One q-tile of attention for both heads of batch b."""
                q_sl = slice(qt * QTILE, (qt + 1) * QTILE)
                accs = [
                    accp.tile([128, QTILE], F32, tag="acc", name="acc")
                    for _ in range(HPC)
                ]
                for j in range(NKT):
                    k_sl = slice(j * KTILE, (j + 1) * KTILE)
                    sp = spsum.tile([128, HPC, QTILE], F32, tag="s")
                    for h in range(HPC):
                        h_sl = slice(h * DH, (h + 1) * DH)
                        nc.tensor.matmul(
                            sp[:, h, :],
                            kt_sb[b][h_sl, k_sl],
                            qt_sb[b][h_sl, q_sl],
                            start=True,
                            stop=True,
                        )
                    ex = exps.tile([128, HPC, QTILE], BF16, tag="e")
                    nc.scalar.activation(ex[:], sp[:], Exp, scale=SCALE)
                    for h in range(HPC):
                        nc.tensor.matmul(
                            accs[h][0 : DH + 1, :],
                            vnat[b][:, h, j, :],
                            ex[:, h, :],
                            start=(j == 0),
                            stop=(j == NKT - 1),
                        )
                for h in range(HPC):
                    # free the PSUM accumulator immediately; normalization
                    # happens later in norm_flush
                    nc.vector.tensor_copy(
                        usb[b][:, qt * HPC + h, :], accs[h][0 : DH + 1, :]
                    )

            def outproj(b, tt0, tt1):
                for tt in range(tt0, tt1):
                    t_sl = slice(tt * 128, (tt + 1) * 128)
                    for nt in range(DIM // 512):
                        ps = miscp.tile([128, 512], F32, tag="m", name="projo")
                        nc.tensor.matmul(
                            ps[:],
                            outT[b][:, t_sl],
                            wout_sb[:, nt * 512 : (nt + 1) * 512],
                            start=True,
                            stop=True,
                        )
                        ob = ost.tile([128, 512], BF16, tag="o")
                        nc.vector.tensor_copy(ob[:], ps[:])
                        nc.gpsimd.dma_start(
                            out_d.ap()[
                                b * SQ + tt * 128 : b * SQ + (tt + 1) * 128,
                                nt * 512 : (nt + 1) * 512,
                            ],
                            ob[:],
                        )

            def kv_chunk(b, tt):
                """Load one x_kv chunk once; project both K and V from it,
                then build the matching vnat groups."""
                xt = load_chunk(xkvt, b * SKV, tt)
                _proj(kt_sb[b], wk_sb, xt, tt)
                _proj(vt_sb[b], wv_sb, xt, tt)
                vnat_group(b, 2 * tt)
                vnat_group(b, 2 * tt + 1)

            def qkv_pieces(b):
                """Generator of fine-grained projection emission steps."""
                yield lambda: kv_chunk(b, 0)
                yield lambda: proj_chunk(qt_sb[b], wq_sb, xqt, b * SQ, 0)
                yield lambda: kv_chunk(b, 1)
                yield lambda: proj_chunk(qt_sb[b], wq_sb, xqt, b * SQ, 1)

            # --- emission schedule: fine-grained interleave so the scheduler
            # always has dep-free PE work to fill ACT-bound attention gaps ---
            for piece in qkv_pieces(0):
                piece()

            nxt = qkv_pieces(1)

            def emit_next(n):
                for _ in range(n):
                    p = next(nxt, None)
                    if p is not None:
                        p()

            # flush(b, qt) normalizes the two units of q-tile qt;
            # outproj quarter (4 token-tiles) trails each flush
            attn_qt(0, 0)
            emit_next(1)  # kv(b1) c0
            attn_qt(0, 1)
            norm_flush(0, 0, HPC)
            outproj(0, 0, 4)
            emit_next(1)  # q(b1) c0
            attn_qt(0, 2)
            norm_flush(0, HPC, HPC)
            outproj(0, 4, 8)
            emit_next(1)  # kv(b1) c1
            attn_qt(0, 3)
            norm_flush(0, 2 * HPC, HPC)
            outproj(0, 8, 12)
            emit_next(1)  # q(b1) c1
            norm_flush(0, 3 * HPC, HPC)
            outproj(0, 12, 16)

            attn_qt(1, 0)
            attn_qt(1, 1)
            norm_flush(1, 0, HPC)
            outproj(1, 0, 4)
            attn_qt(1, 2)
            norm_flush(1, HPC, HPC)
            outproj(1, 4, 8)
            attn_qt(1, 3)
            norm_flush(1, 2 * HPC, HPC)
            outproj(1, 8, 12)
            norm_flush(1, 3 * HPC, HPC)
            outproj(1, 12, 16)

    nc.compile()
    return nc


def make_in_maps(x_q, x_kv, W_qkv, W_out):
    x_q = np.asarray(x_q, dtype=np.float32)
    x_kv = np.asarray(x_kv, dtype=np.float32)
    W_qkv = np.asarray(W_qkv, dtype=np.float32)
    W_out = np.asarray(W_out, dtype=np.float32)

    xqt = np.ascontiguousarray(x_q.reshape(TOK, DIM).T).astype(BF)
    xkvt = np.ascontiguousarray(x_kv.reshape(TOK, DIM).T).astype(BF)

    in_maps = []
    for c in range(N_CORES):
        cs = slice(c * HD, (c + 1) * HD)
        in_maps.append(
            {
                "xqt": xqt,
                "xkvt": xkvt,
                "wq": np.ascontiguousarray(W_qkv[:, cs]).astype(BF),
                "wk": np.ascontiguousarray(W_qkv[:, 1024:][:, cs]).astype(BF),
                "wv": np.ascontiguousarray(W_qkv[:, 2048:][:, cs]).astype(BF),
                "wout": np.ascontiguousarray(W_out[cs, :]).astype(BF),
            }
        )
    return in_maps


def combine(partials, b_out):
    """Sum the 8 per-core partial projections and add the bias."""
    acc = np.zeros((TOK, DIM), dtype=np.float32)
    for p in partials:
        acc += np.asarray(p, dtype=np.float32)
    acc += np.asarray(b_out, dtype=np.float32)
    return acc.reshape(B, SQ, DIM)


_STATE = {}


def _get_nc():
    if "nc" not in _STATE:
        _STATE["nc"] = build()
    return _STATE["nc"]


def run(x_q, x_kv, W_qkv, W_out, b_out, trace=False):
    nc = _get_nc()
    in_maps = make_in_maps(x_q, x_kv, W_qkv, W_out)
    res = run_bass_kernel_spmd(nc, in_maps, list(range(N_CORES)), trace=trace)
    out = combine([r["out"] for r in res.results], b_out)
    return out, res


def kernel(x_q, x_kv, W_qkv, W_out, b_out):
    out, _ = run(x_q, x_kv, W_qkv, W_out, b_out, trace=False)
    return out


# revision 18
# speedup vs baseline: 1.6102x; 1.0555x over previous
"""Multi-head attention (b=2, sq=skv=2048, dim=1024, 16 heads x 64) on 8 TRN2
NeuronCores.

Sharding: 2 heads per core (head-parallel across batch*heads), with the
matching tensor-parallel column slice of W_qkv and row slice of W_out.  Each
core computes a partial output projection over its 128 head-dims; the
all-reduce of the 8 partials (+ bias) happens on the host during unshard.

Per-core kernel (bf16 compute, fp32 PSUM accumulation):
  phase 1: qT/kT/vT = W.T @ x.T   ([128 = 2 heads x 64 dims, tokens]); v is
           additionally PE-transposed to natural [token, dim] layout with a
           ones column appended (denominator trick).
  phase 2: per (batch, q-tile, k-tile): scoresT for both heads ([k-tokens, q])
           in one 2-bank PSUM group; one exp ACTIVATE over the group (scale
           1/8 fused, no max subtraction -- scores range +-10); PV matmuls
           accumulate [v | 1].T @ expT over the 16 k-tiles giving unnormalized
           outT plus the softmax denominator in row 64.  The accumulator is
           copied to SBUF immediately (releasing PSUM); normalization
           (reciprocal + PE outer-product broadcast + multiply) happens off
           the critical path.
  phase 3: partial out = outT.T @ W_out_rows, DMA'd straight from PSUM to
           DRAM as fp32 [tokens, 1024].
"""

import os
import sys

for _p in ("/opt/trn_rl_repo", "/root/.axon_site/_ro/trn_rl_repo"):
    if os.path.isdir(_p) and _p not in sys.path:
        sys.path.append(_p)

import ml_dtypes
import numpy as np

import concourse.bass as bass  # noqa: F401
import concourse.tile as tile
from concourse import bacc, mybir
from concourse.bass_utils import run_bass_kernel_spmd
from concourse.masks import make_identity

B, SQ, SKV, DIM = 2, 2048, 2048, 1024
HEADS, DH = 16, 64
N_CORES = 8
HPC = HEADS // N_CORES  # heads per core = 2
HD = HPC * DH  # 128 head-dim rows per core
TOK = B * SQ  # 4096
KO = DIM // 128  # 8 contraction chunks of 128
SCALE = DH**-0.5

BF16 = mybir.dt.bfloat16
F32 = mybir.dt.float32

PCHUNK = 1024  # token chunk in projections (2KB dram lines)
QTILE = 512  # q tile in attention
KTILE = 128  # k tile (scores psum partition dim)
NKT = SKV // KTILE  # 16
NQT = SQ // QTILE  # 4

BF = ml_dtypes.bfloat16
Exp = mybir.ActivationFunctionType.Exp


def build():
    nc = bacc.Bacc(
        "TRN2", target_bir_lowering=False, debug=False, num_devices=N_CORES
    )

    xqt_d = nc.dram_tensor("xqt", [DIM, TOK], BF16, kind="ExternalInput")
    xkvt_d = nc.dram_tensor("xkvt", [DIM, TOK], BF16, kind="ExternalInput")
    wq_d = nc.dram_tensor("wq", [DIM, HD], BF16, kind="ExternalInput")
    wk_d = nc.dram_tensor("wk", [DIM, HD], BF16, kind="ExternalInput")
    wv_d = nc.dram_tensor("wv", [DIM, HD], BF16, kind="ExternalInput")
    wout_d = nc.dram_tensor("wout", [HD, DIM], BF16, kind="ExternalInput")
    out_d = nc.dram_tensor("out", [TOK, DIM], BF16, kind="ExternalOutput")

    xqt = xqt_d.ap().rearrange("(ko p) t -> p ko t", p=128)
    xkvt = xkvt_d.ap().rearrange("(ko p) t -> p ko t", p=128)

    with tile.TileContext(nc) as tc:
        with (
            tc.tile_pool(name="persist", bufs=1) as persist,
            tc.tile_pool(name="xin", bufs=3) as xin,
            tc.tile_pool(name="exps", bufs=6) as exps,
            tc.tile_pool(name="ost", bufs=3) as ost,
            tc.tile_pool(name="spsum", bufs=2, space="PSUM") as spsum,
            tc.tile_pool(name="accp", bufs=2, space="PSUM") as accp,
            tc.tile_pool(name="miscp", bufs=2, space="PSUM") as miscp,
            tc.tile_pool(name="drp", bufs=2, space="DRAM") as drp,
        ):
            # --- weights / constants ---
            wq_sb = persist.tile([128, KO, HD], BF16, tag="wq")
            nc.sync.dma_start(wq_sb[:], wq_d.ap().rearrange("(ko p) m -> p ko m", p=128))
            wk_sb = persist.tile([128, KO, HD], BF16, tag="wk")
            nc.sync.dma_start(wk_sb[:], wk_d.ap().rearrange("(ko p) m -> p ko m", p=128))
            wv_sb = persist.tile([128, KO, HD], BF16, tag="wv")
            nc.sync.dma_start(wv_sb[:], wv_d.ap().rearrange("(ko p) m -> p ko m", p=128))
            wout_sb = persist.tile([HD, DIM], BF16, tag="wout")
            nc.sync.dma_start(wout_sb[:], wout_d.ap())

            ident = persist.tile([128, DH], BF16, tag="ident")
            make_identity(nc, ident[0:DH, :])
            make_identity(nc, ident[DH : 2 * DH, :])
            ones_f32 = persist.tile([1, DH], F32, tag="ones")
            nc.vector.memset(ones_f32[:], 1.0)

            qt_sb, kt_sb, vt_sb, vnat, outT, usb = {}, {}, {}, {}, {}, {}
            for b in range(B):
                qt_sb[b] = persist.tile([HD, SQ], BF16, tag=f"qt{b}", name=f"qt{b}")
                kt_sb[b] = persist.tile([HD, SKV], BF16, tag=f"kt{b}", name=f"kt{b}")
                vt_sb[b] = persist.tile([HD, SKV], BF16, tag=f"vt{b}", name=f"vt{b}")
                vnat[b] = persist.tile(
                    [128, HPC, NKT, DH + 1], BF16, tag=f"vn{b}", name=f"vn{b}"
                )
                outT[b] = persist.tile([HD, SQ], BF16, tag=f"ot{b}", name=f"ot{b}")
                # unnormalized outT + denominators, unit index = qt*HPC + h
                usb[b] = persist.tile(
                    [DH + 1, NQT * HPC, QTILE], F32, tag=f"us{b}", name=f"us{b}"
                )
                nc.vector.memset(vnat[b][:, :, :, DH], 1.0)

            def _proj(dst, w_sb, xt, tt):
                for sub in range(PCHUNK // 512):
                    ps = miscp.tile([128, 512], F32, tag="m", name="projp")
                    for ko in range(KO):
                        nc.tensor.matmul(
                            ps[:],
                            w_sb[:, ko, :],
                            xt[:, ko, sub * 512 : (sub + 1) * 512],
                            start=(ko == 0),
                            stop=(ko == KO - 1),
                        )
                    t0 = tt * PCHUNK + sub * 512
                    nc.vector.tensor_copy(dst[:, t0 : t0 + 512], ps[:])

            def load_chunk(x_ap, tok0, tt):
                xt = xin.tile([128, KO, PCHUNK], BF16, tag="x")
                nc.sync.dma_start(
                    xt[:],
                    x_ap[:, :, tok0 + tt * PCHUNK : tok0 + (tt + 1) * PCHUNK],
                )
                return xt

            def proj_chunk(dst, w_sb, x_ap, tok0, tt):
                """Project one PCHUNK of tokens into dst[:, tt*PCHUNK...]."""
                _proj(dst, w_sb, load_chunk(x_ap, tok0, tt), tt)

            def vnat_group(b, jg):
                """PE-transpose k-tiles 4jg..4jg+3 of vT into natural layout."""
                for h in range(HPC):
                    tp = miscp.tile([128, 4, DH], BF16, tag="m", name="vtp")
                    for i in range(4):
                        j = jg * 4 + i
                        nc.tensor.transpose(
                            tp[:, i, :],
                            vt_sb[b][
                                h * DH : (h + 1) * DH,
                                j * KTILE : (j + 1) * KTILE,
                            ],
                            ident[h * DH : (h + 1) * DH, :],
                        )
                    nc.vector.tensor_copy(
                        vnat[b][:, h, jg * 4 : (jg + 1) * 4, 0:DH], tp[:]
                    )

            F32R = mybir.dt.float32r

            def norm_flush(b, u0, nu):
                """Normalize units u0..u0+nu-1 of usb[b] into outT[b].

                Batches the reciprocal: denominator rows are bounced through
                DRAM to repack [1, nu, QTILE] -> [128, nu*QTILE/128] so the
                DVE reciprocal runs wide, then bounced back and broadcast to
                64 partitions with an f32r PE outer product.
                """
                nel = nu * QTILE
                d1 = drp.tile([1, nu, QTILE], F32, tag="d1", name="d1")
                nc.gpsimd.dma_start(d1[:], usb[b][DH : DH + 1, u0 : u0 + nu, :])
                dpk = ost.tile([128, nel // 128], F32, tag="dp", name="dpk")
                nc.gpsimd.dma_start(
                    dpk[:],
                    d1[:]
                    .rearrange("a b c -> (a b c)")
                    .rearrange("(p f) -> p f", p=128),
                )
                rpk = ost.tile([128, nel // 128], F32, tag="rp", name="rpk")
                nc.vector.reciprocal(rpk[:], dpk[:])
                d2 = drp.tile([1, nu, QTILE], F32, tag="d2", name="d2")
                nc.gpsimd.dma_start(
                    d2[:]
                    .rearrange("a b c -> (a b c)")
                    .rearrange("(p f) -> p f", p=128),
                    rpk[:],
                )
                rst = ost.tile([1, nu, QTILE], F32, tag="rs", name="rst")
                nc.gpsimd.dma_start(rst[:], d2[:])
                for i in range(nu):
                    g = u0 + i
                    qt, h = divmod(g, HPC)
                    bc = miscp.tile([DH, QTILE], F32, tag="m", name="bc")
                    nc.tensor.matmul(
                        bc[:],
                        ones_f32[:].bitcast(F32R),
                        rst[0:1, i, :].bitcast(F32R),
                        start=True,
                        stop=True,
                    )
                    nc.vector.tensor_mul(
                        outT[b][h * DH : (h + 1) * DH, qt * QTILE : (qt + 1) * QTILE],
                        usb[b][0:DH, g, :],
                        bc[:],
                    )

            LOOKAHEAD = 2

            def attention(b, hooks):
                """Flat software-pipelined attention over all (qt, j) steps.

                Scores for step t+2 are emitted before PV of step t, so the
                PE always has score matmuls queued ahead of the exp/PV chain
                and q-tile boundaries pipeline seamlessly.  hooks[qt] is a
                list of emission callables run right after qt's accumulators
                are released.
                """
                NT = NQT * NKT
                sps, accs = {}, {}

                def emit_scores(t):
                    qt, j = divmod(t, NKT)
                    q_sl = slice(qt * QTILE, (qt + 1) * QTILE)
                    k_sl = slice(j * KTILE, (j + 1) * KTILE)
                    sp = spsum.tile([128, HPC, QTILE], F32, tag="s", name="sp")
                    sps[t] = sp
                    for h in range(HPC):
                        h_sl = slice(h * DH, (h + 1) * DH)
                        nc.tensor.matmul(
                            sp[:, h, :],
                            kt_sb[b][h_sl, k_sl],
                            qt_sb[b][h_sl, q_sl],
                            start=True,
                            stop=True,
                        )

                def emit_tail(t):
                    qt, j = divmod(t, NKT)
                    sp = sps.pop(t)
                    ex = exps.tile([128, HPC, QTILE], BF16, tag="e", name="ex")
                    nc.scalar.activation(ex[:], sp[:], Exp, scale=SCALE)
                    if j == 0:
                        accs[qt] = [
                            accp.tile([128, QTILE], F32, tag="acc", name="acc")
                            for _ in range(HPC)
                        ]
                    for h in range(HPC):
                        nc.tensor.matmul(
                            accs[qt][h][0 : DH + 1, :],
                            vnat[b][:, h, j, :],
                            ex[:, h, :],
                            start=(j == 0),
                            stop=(j == NKT - 1),
                        )
                    if j == NKT - 1:
                        for h in range(HPC):
                            # free the PSUM accumulator; normalization comes
                            # later in norm_flush
                            nc.vector.tensor_copy(
                                usb[b][:, qt * HPC + h, :],
                                accs[qt][h][0 : DH + 1, :],
                            )
                        del accs[qt]
                        for fn in hooks.get(qt, []):
                            fn()

                for t in range(NT + LOOKAHEAD):
                    if t < NT:
                        emit_scores(t)
                    if t >= LOOKAHEAD:
                        emit_tail(t - LOOKAHEAD)

            def outproj(b, tt0, tt1):
                for tt in range(tt0, tt1):
                    t_sl = slice(tt * 128, (tt + 1) * 128)
                    for nt in range(DIM // 512):
                        ps = miscp.tile([128, 512], F32, tag="m", name="projo")
                        nc.tensor.matmul(
                            ps[:],
                            outT[b][:, t_sl],
                            wout_sb[:, nt * 512 : (nt + 1) * 512],
                            start=True,
                            stop=True,
                        )
                        ob = ost.tile([128, 512], BF16, tag="o")
                        nc.vector.tensor_copy(ob[:], ps[:])
                        nc.gpsimd.dma_start(
                            out_d.ap()[
                                b * SQ + tt * 128 : b * SQ + (tt + 1) * 128,
                                nt * 512 : (nt + 1) * 512,
                            ],
                            ob[:],
                        )

            def qkv_pieces(b):
                """Generator of fine-grained projection emission steps.

                K first so score matmuls can start as early as possible; the
                shared x_kv chunk is loaded once for both K and V.
                """
                xts = {}

                def kv_load_k(tt):
                    xts[tt] = load_chunk(xkvt, b * SKV, tt)
                    _proj(kt_sb[b], wk_sb, xts[tt], tt)

                def v_part(tt):
                    _proj(vt_sb[b], wv_sb, xts.pop(tt), tt)
                    vnat_group(b, 2 * tt)
                    vnat_group(b, 2 * tt + 1)

                yield lambda: kv_load_k(0)
                yield lambda: proj_chunk(qt_sb[b], wq_sb, xqt, b * SQ, 0)
                yield lambda: v_part(0)
                yield lambda: kv_load_k(1)
                yield lambda: proj_chunk(qt_sb[b], wq_sb, xqt, b * SQ, 1)
                yield lambda: v_part(1)

            # --- emission schedule: fine-grained interleave so the scheduler
            # always has dep-free PE work to fill ACT-bound attention gaps ---
            for piece in qkv_pieces(0):
                piece()

            nxt = qkv_pieces(1)

            def emit_next(n):
                def go():
                    for _ in range(n):
                        p = next(nxt, None)
                        if p is not None:
                            p()

                return go

            def flush_op(b, qt):
                def go():
                    norm_flush(b, qt * HPC, HPC)
                    outproj(b, qt * 4, qt * 4 + 4)

                return go

            attention(
                0,
                {
                    0: [emit_next(2)],
                    1: [emit_next(1), flush_op(0, 0)],
                    2: [emit_next(2), flush_op(0, 1)],
                    3: [emit_next(1), flush_op(0, 2)],
                },
            )
            norm_flush(0, 3 * HPC, HPC)
            attention(
                1,
                {
                    0: [lambda: outproj(0, 12, 16)],
                    1: [flush_op(1, 0)],
                    2: [flush_op(1, 1)],
                    3: [flush_op(1, 2)],
                },
            )
            norm_flush(1, 3 * HPC, HPC)
            outproj(1, 12, 16)

    nc.compile()
    return nc


def make_in_maps(x_q, x_kv, W_qkv, W_out):
    x_q = np.asarray(x_q, dtype=np.float32)
    x_kv = np.asarray(x_kv, dtype=np.float32)
    W_qkv = np.asarray(W_qkv, dtype=np.float32)
    W_out = np.asarray(W_out, dtype=np.float32)

    xqt = np.ascontiguousarray(x_q.reshape(TOK, DIM).T).astype(BF)
    xkvt = np.ascontiguousarray(x_kv.reshape(TOK, DIM).T).astype(BF)

    in_maps = []
    for c in range(N_CORES):
        cs = slice(c * HD, (c + 1) * HD)
        in_maps.append(
            {
                "xqt": xqt,
                "xkvt": xkvt,
                "wq": np.ascontiguousarray(W_qkv[:, cs]).astype(BF),
                "wk": np.ascontiguousarray(W_qkv[:, 1024:][:, cs]).astype(BF),
                "wv": np.ascontiguousarray(W_qkv[:, 2048:][:, cs]).astype(BF),
                "wout": np.ascontiguousarray(W_out[cs, :]).astype(BF),
            }
        )
    return in_maps


def combine(partials, b_out):
    """Sum the 8 per-core partial projections and add the bias."""
    acc = np.zeros((TOK, DIM), dtype=np.float32)
    for p in partials:
        acc += np.asarray(p, dtype=np.float32)
    acc += np.asarray(b_out, dtype=np.float32)
    return acc.reshape(B, SQ, DIM)


_STATE = {}


def _get_nc():
    if "nc" not in _STATE:
        _STATE["nc"] = build()
    return _STATE["nc"]


def run(x_q, x_kv, W_qkv, W_out, b_out, trace=False):
    nc = _get_nc()
    in_maps = make_in_maps(x_q, x_kv, W_qkv, W_out)
    res = run_bass_kernel_spmd(nc, in_maps, list(range(N_CORES)), trace=trace)
    out = combine([r["out"] for r in res.results], b_out)
    return out, res


def kernel(x_q, x_kv, W_qkv, W_out, b_out):
    out, _ = run(x_q, x_kv, W_qkv, W_out, b_out, trace=False)
    return out
